# revision 15
# baseline (speedup 1.0000x reference)
"""Trainium2 Bass kernel for nn_Block_26628797235524 (Mamba-style cross-scan SSM block).

Sharding: batch B=8 -> one batch element per NeuronCore (SPMD, no collectives).

v2 design (vs v1 baseline at ~2.2ms):
  - bf16 dataflow everywhere precision allows (matmuls, elementwise, scan
    inputs/outputs); the scan's dA decay factor stays fp32 (tensor_tensor_scan
    keeps fp32 internal state, and exp(A*delta)~1 would be wrecked by bf16).
  - Silu activation function on the Scalar engine (1 op instead of
    sigmoid+2 vector multiplies).
  - Full-L (4096) scans: no slab chaining, no state copies.
  - Skip connection Ds*xs injected as a diag(Ds) matmul directly into the
    PSUM y accumulator (start=True), freeing the vector engine.
  - xs orderings are APs (strided / reversed views of xh), never materialized.
  - z and B/C rows spilled to DRAM in bf16; B/C broadcast via DMA
    partition_broadcast per state-dim n.
  - All elementwise scan work on DVE (Pool's bf16 reads are slow and it
    shares SBUF ports with DVE); Scalar does all exp/silu/softplus/copies.
"""

import os
import sys
from contextlib import ExitStack

import numpy as np
import ml_dtypes

_RL = "/opt/trn_rl_repo"
if os.path.isdir(_RL) and _RL not in sys.path:
    sys.path.insert(0, _RL)

import concourse.bass as bass
import concourse.bacc as bacc
import concourse.tile as tile
from concourse import mybir
from concourse.bass_utils import run_bass_kernel_spmd

B, T, H, W, DIM = 8, 16, 16, 16, 128
DIN, DST, DTR, KG = 256, 16, 8, 2
L = T * H * W          # 4096
P = 128
HWC = H * W            # 256
DH = DIN // P          # 2
NCORES = 8
MM_F = 512             # PSUM bank: max matmul free dim (fp32 out)
NMM = L // MM_F        # 8

F32 = mybir.dt.float32
BF16 = mybir.dt.bfloat16
XIN_ON_POOL = bool(int(os.environ.get("XIN_ON_POOL", "0")))
AF = mybir.ActivationFunctionType
ALU = mybir.AluOpType


def _declare_drams(nc):
    d = {}

    def inp(name, shape, dt=BF16):
        d[name] = nc.dram_tensor(name, list(shape), dt, kind="ExternalInput")

    inp("xTb", (P, L))                      # per-core x, channel-major, bf16
    inp("w_in", (P, 4 * P))                 # in_proj_w.T
    inp("conv_sc", (DH, P, 1), F32)
    inp("conv_bi", (DH, P, 1), F32)
    inp("w_xproj", (KG, DH, P, 40))         # x_proj_w[k].T per d-half
    inp("w_dt", (KG, DTR, DIN))             # dt_w[k].T
    inp("dt_bias", (KG, DH, P, 1), F32)
    inp("a_mat", (KG, DH, P, DST), F32)     # A = -exp(A_logs)
    inp("diag_ds", (KG, DH, P, P))          # diag(Ds) per (k, d-half)
    inp("ident", (P, P))
    inp("ones_col", (P, 1))
    inp("one_f32", (P, 1), F32)
    inp("lnw", (DH, P, 1), F32)
    inp("lnb", (DH, P, 1), F32)
    inp("w_out", (DH, P, P))                # out_proj_w.T per d-half
    inp("inv_din", (1, 1), F32)             # 1/256
    inp("neg_one", (1, 1), F32)
    inp("eps11", (1, 1), F32)
    d["z_dram"] = nc.dram_tensor("z_dram", [DH, P, L], BF16)
    d["bc_dram"] = nc.dram_tensor("bc_dram", [KG, 2 * DST, L], BF16)
    d["mr_dram"] = nc.dram_tensor("mr_dram", [2, L], BF16)
    d["outT"] = nc.dram_tensor("outT", [P, L], F32, kind="ExternalOutput")
    return d


def _body(tc, d):
    nc = tc.nc
    with ExitStack() as ctx:
        const = ctx.enter_context(tc.tile_pool(name="const", bufs=1))

        def cload(name, shape, dt=BF16, src=None):
            t = const.tile(list(shape), dt, tag=name, name=name)
            nc.sync.dma_start(t[:], src if src is not None else d[name][:])
            return t

        w_in = cload("w_in", (P, 4 * P))
        conv_sc = [cload(f"conv_sc{i}", (P, 1), F32, d["conv_sc"][i]) for i in range(DH)]
        conv_bi = [cload(f"conv_bi{i}", (P, 1), F32, d["conv_bi"][i]) for i in range(DH)]
        w_xproj = [[cload(f"w_xproj{k}{i}", (P, 40), BF16, d["w_xproj"][k, i])
                    for i in range(DH)] for k in range(KG)]
        w_dt = [cload(f"w_dt{k}", (DTR, DIN), BF16, d["w_dt"][k]) for k in range(KG)]
        dt_bias = [[cload(f"dt_bias{k}{i}", (P, 1), F32, d["dt_bias"][k, i])
                    for i in range(DH)] for k in range(KG)]
        a_mat = [[cload(f"a_mat{k}{i}", (P, DST), F32, d["a_mat"][k, i])
                  for i in range(DH)] for k in range(KG)]
        diag_ds = [[cload(f"diag_ds{k}{i}", (P, P), BF16, d["diag_ds"][k, i])
                    for i in range(DH)] for k in range(KG)]
        ident = cload("ident", (P, P))
        ones_col = cload("ones_col", (P, 1))
        one_f32 = cload("one_f32", (P, 1), F32)
        lnw = [cload(f"lnw{i}", (P, 1), F32, d["lnw"][i]) for i in range(DH)]
        lnb = [cload(f"lnb{i}", (P, 1), F32, d["lnb"][i]) for i in range(DH)]
        w_out = [cload(f"w_out{i}", (P, P), BF16, d["w_out"][i]) for i in range(DH)]
        inv_din = cload("inv_din", (1, 1), F32)
        neg_one = cload("neg_one", (1, 1), F32)
        eps11 = cload("eps11", (1, 1), F32)

        pers = ctx.enter_context(tc.tile_pool(name="pers", bufs=1))
        xh = [pers.tile([P, L], BF16, tag=f"xh{i}", name=f"xh{i}") for i in range(DH)]
        delta = [[pers.tile([P, L], BF16, tag=f"dl{k}{i}", name=f"dl{k}{i}")
                  for i in range(DH)] for k in range(KG)]
        y_sb = [[pers.tile([P, L], BF16, tag=f"y{k}{i}", name=f"y{k}{i}")
                 for i in range(DH)] for k in range(KG)]

        # xs views: k=0 spectral order (hw, t); k=1 reversed natural order.
        def xs_full(i, k):
            if k == 0:
                return xh[i][:].rearrange("p (t hw) -> p hw t", t=T, hw=HWC)
            return xh[i][:, ::-1]

        def xs_chunk(i, k, c):
            if k == 0:
                v = xh[i][:].rearrange("p (t hw) -> p hw t", t=T, hw=HWC)
                nh = MM_F // T  # 32 hw values per 512-col chunk
                return v[:, c * nh:(c + 1) * nh, :]
            hi = L - 1 - c * MM_F
            lo = L - (c + 1) * MM_F - 1
            return xh[i][:, hi:(None if lo < 0 else lo):-1]

        # ================= Phase A: in_proj + depthwise conv + silu ==========
        with tc.tile_pool(name="pA", bufs=1) as pA, \
             tc.tile_pool(name="psA", bufs=4, space=bass.MemorySpace.PSUM) as psA:
            xTb = pA.tile([P, L], BF16, tag="xTb", name="xTb")
            nc.sync.dma_start(xTb[:], d["xTb"][:])
            for po in range(4):
                for c in range(NMM):
                    cs = slice(c * MM_F, (c + 1) * MM_F)
                    ps = psA.tile([P, MM_F], F32, tag="pa", name="pa")
                    nc.tensor.matmul(ps[:], w_in[:, po * P:(po + 1) * P],
                                     xTb[:, cs], start=True, stop=True)
                    if po < DH:
                        nc.scalar.activation(xh[po][:, cs], ps[:], AF.Silu,
                                             bias=conv_bi[po][:], scale=conv_sc[po][:])
                    else:
                        zc = pA.tile([P, MM_F], BF16, tag="zc", name="zc", bufs=3)
                        nc.scalar.activation(zc[:], ps[:], AF.Silu)
                        nc.sync.dma_start(d["z_dram"][po - DH, :, cs], zc[:])

        # ================= Phase B: x_dbl -> B/C spill, delta ================
        with tc.tile_pool(name="pB", bufs=1) as pB, \
             tc.tile_pool(name="psB", bufs=2, space=bass.MemorySpace.PSUM) as psB:
            for k in range(KG):
                xdbl = pB.tile([40, L], BF16, tag="xdbl", name=f"xdbl{k}", bufs=2)
                for c in range(NMM):
                    cs = slice(c * MM_F, (c + 1) * MM_F)
                    ps = psB.tile([40, MM_F], F32, tag="pb", name="pb")
                    nc.tensor.matmul(ps[:], w_xproj[k][0][:], xs_chunk(0, k, c),
                                     start=True, stop=False)
                    nc.tensor.matmul(ps[:], w_xproj[k][1][:], xs_chunk(1, k, c),
                                     start=False, stop=True)
                    nc.scalar.activation(xdbl[:, cs], ps[:], AF.Copy)
                nc.sync.dma_start(d["bc_dram"][k, :, 0:L // 2], xdbl[DTR:40, 0:L // 2])
                nc.sync.dma_start(d["bc_dram"][k, :, L // 2:L],
                                  xdbl[DTR:40, L // 2:L])
                for i in range(DH):
                    # softplus(x + dt_b) = ln(1 + exp(x + dt_b)); batch the 8
                    # Exp chunks then one full-width Ln so the Exp/Ln act
                    # tables load once each instead of alternating.
                    ed = pB.tile([P, L], F32, tag="ed", name="ed", bufs=1)
                    for c in range(NMM):
                        cs = slice(c * MM_F, (c + 1) * MM_F)
                        ps2 = psB.tile([P, MM_F], F32, tag="pb2", name="pb2")
                        nc.tensor.matmul(ps2[:], w_dt[k][:, i * P:(i + 1) * P],
                                         xdbl[0:DTR, cs], start=True, stop=True)
                        nc.scalar.activation(ed[:, cs], ps2[:], AF.Exp,
                                             bias=dt_bias[k][i][:])
                    nc.scalar.activation(delta[k][i][:], ed[:], AF.Ln,
                                         bias=one_f32[:])

        # ================= Phase C: selective scan ===========================
        # Half-L slabs: smaller tiles allow 4-deep broadcast prefetch (the
        # full-L version stalled the scan ~10us/iter on brep/crep DMAs), and
        # the two d-halves interleave per n. State chains across halves via a
        # first-column fixup (xin[0] += dA[0]*state) so `initial` stays 0.0.
        LC = L // 2
        NMC = LC // MM_F
        with tc.tile_pool(name="sc", bufs=2) as sc, \
             tc.tile_pool(name="psC", bufs=1, space=bass.MemorySpace.PSUM) as psC:
            state = sc.tile([P, 2 * DST], F32, tag="state", name="state", bufs=1)
            for k in range(KG):
                dUs = []
                for i in range(DH):
                    dU = sc.tile([P, L], F32 if XIN_ON_POOL else BF16,
                                 tag=f"dU{i}", name=f"dU{i}", bufs=1)
                    if k == 0:
                        nc.vector.tensor_tensor(
                            dU[:].rearrange("p (hw t) -> p hw t", hw=HWC, t=T),
                            delta[k][i][:].rearrange("p (hw t) -> p hw t", hw=HWC, t=T),
                            xs_full(i, k), ALU.mult)
                    else:
                        nc.vector.tensor_tensor(dU[:], delta[k][i][:],
                                                xs_full(i, k), ALU.mult)
                    dUs.append(dU)
                for half in range(2):
                    hs = slice(half * LC, (half + 1) * LC)
                    y_ps = [psC.tile([P, LC], F32, tag=f"yps{i}", name=f"yps{i}")
                            for i in range(DH)]
                    for i in range(DH):
                        for c in range(NMC):
                            nc.tensor.matmul(
                                y_ps[i][:, c * MM_F:(c + 1) * MM_F],
                                diag_ds[k][i][:],
                                xs_chunk(i, k, half * NMC + c),
                                start=True, stop=False)
                    for n in range(DST):
                        brep = sc.tile([P, LC], BF16, tag="brep", name="brep",
                                       bufs=5)
                        nc.sync.dma_start(
                            brep[:],
                            d["bc_dram"][k, n:n + 1, hs].partition_broadcast(P))
                        crep = sc.tile([P, LC], BF16, tag="crep", name="crep",
                                       bufs=5)
                        nc.sync.dma_start(
                            crep[:],
                            d["bc_dram"][k, DST + n:DST + n + 1,
                                         hs].partition_broadcast(P))
                        for i in range(DH):
                            col = 2 * n + i
                            dA = sc.tile([P, LC], F32, tag="dA", name="dA", bufs=3)
                            nc.scalar.activation(dA[:], delta[k][i][:, hs],
                                                 AF.Exp,
                                                 scale=a_mat[k][i][:, n:n + 1])
                            xin = sc.tile([P, LC], BF16, tag="xin", name="xin")
                            xin_eng = nc.gpsimd if XIN_ON_POOL else nc.vector
                            xin_eng.tensor_tensor(xin[:], dUs[i][:, hs],
                                                  brep[:], ALU.mult)
                            h = sc.tile([P, LC], BF16, tag="h", name="h")
                            nc.vector.tensor_tensor_scan(
                                h[:], dA[:], xin[:],
                                state[:, col:col + 1] if half == 1 else 0.0,
                                ALU.mult, ALU.add)
                            if half == 0:
                                nc.vector.tensor_copy(state[:, col:col + 1],
                                                      h[:, LC - 1:LC])
                            tmp = sc.tile([P, LC], BF16, tag="tmp", name="tmp", bufs=3)
                            nc.vector.tensor_tensor(tmp[:], crep[:], h[:],
                                                    ALU.mult)
                            for c in range(NMC):
                                cs = slice(c * MM_F, (c + 1) * MM_F)
                                nc.tensor.matmul(y_ps[i][:, cs], ident[:],
                                                 tmp[:, cs], start=False,
                                                 stop=(n == DST - 1))
                    for i in range(DH):
                        nc.scalar.activation(y_sb[k][i][:, hs], y_ps[i][:],
                                             AF.Copy)

        # ================= Phase D: combine + LN + gate + out_proj ===========
        with tc.tile_pool(name="pD", bufs=1) as pD, \
             tc.tile_pool(name="psD", bufs=2, space=bass.MemorySpace.PSUM) as psD:
            ysum = [pD.tile([P, L], BF16, tag=f"ys{i}", name=f"ys{i}")
                    for i in range(DH)]
            for i in range(DH):
                y0v = y_sb[0][i][:].rearrange("p (hw t) -> p t hw", hw=HWC, t=T)
                y1v = y_sb[1][i][:, ::-1].rearrange("p (t hw) -> p t hw", t=T, hw=HWC)
                dst = ysum[i][:].rearrange("p (t hw) -> p t hw", t=T, hw=HWC)
                nc.vector.tensor_tensor(dst, y0v, y1v, ALU.add)

            for c in range(NMM):
                cs = slice(c * MM_F, (c + 1) * MM_F)
                ps1 = psD.tile([1, MM_F], F32, tag="ps1", name="ps1")
                nc.tensor.matmul(ps1[:], ones_col[:], ysum[0][:, cs],
                                 start=True, stop=False)
                nc.tensor.matmul(ps1[:], ones_col[:], ysum[1][:, cs],
                                 start=False, stop=True)
                ps2 = psD.tile([1, MM_F], F32, tag="ps2", name="ps2")
                for i in range(DH):
                    yq = pD.tile([P, MM_F], BF16, tag="yq", name="yq", bufs=2)
                    nc.scalar.activation(yq[:], ysum[i][:, cs], AF.Square)
                    nc.tensor.matmul(ps2[:], ones_col[:], yq[:],
                                     start=(i == 0), stop=(i == DH - 1))
                mu = pD.tile([1, MM_F], F32, tag="mu", name="mu", bufs=2)
                nc.scalar.activation(mu[:], ps1[:], AF.Identity, scale=inv_din[:])
                e2 = pD.tile([1, MM_F], F32, tag="e2", name="e2", bufs=2)
                nc.scalar.activation(e2[:], ps2[:], AF.Identity, scale=inv_din[:])
                m2 = pD.tile([1, MM_F], F32, tag="m2", name="m2", bufs=2)
                nc.scalar.activation(m2[:], mu[:], AF.Square)
                var = pD.tile([1, MM_F], F32, tag="var", name="var", bufs=2)
                nc.vector.tensor_tensor(var[:], e2[:], m2[:], ALU.subtract)
                sd = pD.tile([1, MM_F], F32, tag="sd", name="sd", bufs=2)
                nc.scalar.activation(sd[:], var[:], AF.Sqrt, bias=eps11[:])
                rr = pD.tile([1, MM_F], F32, tag="rr", name="rr", bufs=2)
                nc.vector.reciprocal_approx_fast(rr[:], sd[:])
                a_row = pD.tile([1, MM_F], BF16, tag="a_row", name="a_row", bufs=2)
                nc.scalar.activation(a_row[:], rr[:], AF.Copy)
                t1 = pD.tile([1, MM_F], F32, tag="t1", name="t1", bufs=2)
                nc.vector.tensor_tensor(t1[:], mu[:], rr[:], ALU.mult)
                b_row = pD.tile([1, MM_F], BF16, tag="b_row", name="b_row", bufs=2)
                nc.scalar.activation(b_row[:], t1[:], AF.Identity, scale=neg_one[:])
                nc.sync.dma_start(d["mr_dram"][0:1, cs], a_row[:])
                nc.sync.dma_start(d["mr_dram"][1:2, cs], b_row[:])
            arep = pD.tile([P, L], BF16, tag="arep", name="arep")
            nc.sync.dma_start(arep[:], d["mr_dram"][0:1, :].partition_broadcast(P))
            brep_ln = pD.tile([P, L], BF16, tag="brepl", name="brepl")
            nc.sync.dma_start(brep_ln[:], d["mr_dram"][1:2, :].partition_broadcast(P))

            for c in range(NMM):
                cs = slice(c * MM_F, (c + 1) * MM_F)
                out_ps = psD.tile([P, MM_F], F32, tag="ops", name="ops")
                for i in range(DH):
                    zc = pD.tile([P, MM_F], BF16, tag="zc2", name="zc2", bufs=3)
                    nc.sync.dma_start(zc[:], d["z_dram"][i, :, cs])
                    yn = pD.tile([P, MM_F], BF16, tag="yn", name="yn", bufs=2)
                    nc.vector.tensor_tensor(yn[:], ysum[i][:, cs], arep[:, cs],
                                            ALU.mult)
                    yn2 = pD.tile([P, MM_F], BF16, tag="yn2", name="yn2", bufs=2)
                    nc.vector.tensor_tensor(yn2[:], yn[:], brep_ln[:, cs], ALU.add)
                    ya = pD.tile([P, MM_F], BF16, tag="ya", name="ya", bufs=2)
                    nc.scalar.activation(ya[:], yn2[:], AF.Identity,
                                         bias=lnb[i][:], scale=lnw[i][:])
                    g = pD.tile([P, MM_F], BF16, tag="g", name="g", bufs=2)
                    nc.vector.tensor_tensor(g[:], ya[:], zc[:], ALU.mult)
                    nc.tensor.matmul(out_ps[:], w_out[i][:], g[:],
                                     start=(i == 0), stop=(i == DH - 1))
                osb = pD.tile([P, MM_F], F32, tag="osb", name="osb", bufs=2)
                nc.scalar.activation(osb[:], out_ps[:], AF.Copy)
                nc.sync.dma_start(d["outT"][:, cs], osb[:])


_CACHE = {}


def _get_program():
    if "nc" not in _CACHE:
        nc = bacc.Bacc("TRN2", target_bir_lowering=False, debug=False,
                       num_devices=NCORES)
        d = _declare_drams(nc)
        with tile.TileContext(nc) as tc:
            _body(tc, d)
        nc.compile()
        _CACHE["nc"] = nc
    return _CACHE["nc"]


def _host_weights(inputs):
    f32 = lambda a: np.ascontiguousarray(np.asarray(a, np.float32))
    bf = lambda a: np.ascontiguousarray(np.asarray(a, np.float32)).astype(ml_dtypes.bfloat16)
    in_proj_w = f32(inputs["in_proj_w"])            # (512, 128)
    x_proj_w = f32(inputs["x_proj_w"])              # (2, 40, 256)
    dt_w = f32(inputs["dt_w"])                      # (2, 256, 8)
    dt_b = f32(inputs["dt_b"])                      # (2, 256)
    A_logs = f32(inputs["A_logs"])                  # (512, 16)
    Ds = f32(inputs["Ds"])                          # (512,)
    diag_ds = np.zeros((KG, DH, P, P), np.float32)
    for k in range(KG):
        for i in range(DH):
            np.fill_diagonal(diag_ds[k, i], Ds[k * DIN + i * P:k * DIN + (i + 1) * P])
    m = {
        "w_in": bf(in_proj_w.T),
        "conv_sc": f32(inputs["conv_w"]).reshape(DH, P, 1),
        "conv_bi": f32(inputs["conv_b"]).reshape(DH, P, 1),
        "w_xproj": bf(x_proj_w.transpose(0, 2, 1).reshape(KG, DH, P, 40)),
        "w_dt": bf(dt_w.transpose(0, 2, 1)),
        "dt_bias": f32(dt_b).reshape(KG, DH, P, 1),
        "a_mat": f32(-np.exp(A_logs)).reshape(KG, DH, P, DST),
        "diag_ds": diag_ds.astype(ml_dtypes.bfloat16),
        "ident": np.eye(P, dtype=np.float32).astype(ml_dtypes.bfloat16),
        "ones_col": np.ones((P, 1), np.float32).astype(ml_dtypes.bfloat16),
        "one_f32": np.ones((P, 1), np.float32),
        "lnw": f32(inputs["ln_w"]).reshape(DH, P, 1),
        "lnb": f32(inputs["ln_b"]).reshape(DH, P, 1),
        "w_out": bf(f32(inputs["out_proj_w"]).T.reshape(DH, P, P)),
        "inv_din": np.full((1, 1), 1.0 / DIN, np.float32),
        "neg_one": np.full((1, 1), -1.0, np.float32),
        "eps11": np.full((1, 1), 1e-5, np.float32),
    }
    return m


def kernel(**inputs):
    x = np.ascontiguousarray(np.asarray(inputs["x"], np.float32))   # (8,16,16,16,128)
    shared = _host_weights(inputs)
    nc = _get_program()
    in_maps = []
    for b in range(NCORES):
        m = dict(shared)
        m["xTb"] = np.ascontiguousarray(
            x[b].reshape(L, DIM).T).astype(ml_dtypes.bfloat16)
        in_maps.append(m)
    trace = bool(int(os.environ.get("BASS_PROFILE", "0")))
    res = run_bass_kernel_spmd(nc, in_maps, list(range(NCORES)), trace=trace)
    _CACHE["last_result"] = res
    outs = [np.asarray(r["outT"], np.float32) for r in res.results]
    out = np.stack([o.T.reshape(T, H, W, DIM) for o in outs]).astype(np.float32)
    return out


# revision 19
# speedup vs baseline: 1.0202x; 1.0202x over previous
"""Trainium2 Bass kernel for nn_Block_26628797235524 (Mamba-style cross-scan SSM block).

Sharding: batch B=8 -> one batch element per NeuronCore (SPMD, no collectives).

v2 design (vs v1 baseline at ~2.2ms):
  - bf16 dataflow everywhere precision allows (matmuls, elementwise, scan
    inputs/outputs); the scan's dA decay factor stays fp32 (tensor_tensor_scan
    keeps fp32 internal state, and exp(A*delta)~1 would be wrecked by bf16).
  - Silu activation function on the Scalar engine (1 op instead of
    sigmoid+2 vector multiplies).
  - Full-L (4096) scans: no slab chaining, no state copies.
  - Skip connection Ds*xs injected as a diag(Ds) matmul directly into the
    PSUM y accumulator (start=True), freeing the vector engine.
  - xs orderings are APs (strided / reversed views of xh), never materialized.
  - z and B/C rows spilled to DRAM in bf16; B/C broadcast via DMA
    partition_broadcast per state-dim n.
  - All elementwise scan work on DVE (Pool's bf16 reads are slow and it
    shares SBUF ports with DVE); Scalar does all exp/silu/softplus/copies.
"""

import os
import sys
from contextlib import ExitStack

import numpy as np
import ml_dtypes

_RL = "/opt/trn_rl_repo"
if os.path.isdir(_RL) and _RL not in sys.path:
    sys.path.insert(0, _RL)

import concourse.bass as bass
import concourse.bacc as bacc
import concourse.tile as tile
from concourse import mybir
from concourse.bass_utils import run_bass_kernel_spmd

B, T, H, W, DIM = 8, 16, 16, 16, 128
DIN, DST, DTR, KG = 256, 16, 8, 2
L = T * H * W          # 4096
P = 128
HWC = H * W            # 256
DH = DIN // P          # 2
NCORES = 8
MM_F = 512             # PSUM bank: max matmul free dim (fp32 out)
NMM = L // MM_F        # 8
LC = L // 2            # scan slab length
NMC = LC // MM_F       # 4

F32 = mybir.dt.float32
BF16 = mybir.dt.bfloat16
XIN_ON_POOL = bool(int(os.environ.get("XIN_ON_POOL", "0")))
AF = mybir.ActivationFunctionType
ALU = mybir.AluOpType


def _declare_drams(nc):
    d = {}

    def inp(name, shape, dt=BF16):
        d[name] = nc.dram_tensor(name, list(shape), dt, kind="ExternalInput")

    inp("xTb", (P, L))                      # per-core x, channel-major, bf16
    inp("w_in", (P, 4 * P))                 # in_proj_w.T
    inp("conv_sc", (DH, P, 1), F32)
    inp("conv_bi", (DH, P, 1), F32)
    inp("w_xproj", (KG, DH, P, 40))         # x_proj_w[k].T per d-half
    inp("w_dt", (KG, DTR, DIN))             # dt_w[k].T
    inp("dt_bias", (KG, DH, P, 1), F32)
    inp("a_mat", (KG, DH, P, DST), F32)     # A = -exp(A_logs)
    inp("diag_ds", (KG, DH, P, P))          # diag(Ds) per (k, d-half)
    inp("ident", (P, P))
    inp("ones_col", (P, 1))
    inp("ones_row", (1, P))
    inp("one_f32", (P, 1), F32)
    inp("lnw", (DH, P, 1), F32)
    inp("lnb", (DH, P, 1), F32)
    inp("w_out", (DH, P, P))                # out_proj_w.T per d-half
    inp("inv_din", (1, 1), F32)             # 1/256
    inp("neg_one", (1, 1), F32)
    inp("eps11", (1, 1), F32)
    d["z_dram"] = nc.dram_tensor("z_dram", [DH, P, L], BF16)
    d["bc_dram"] = nc.dram_tensor("bc_dram", [KG, 2, DST, 2, LC], BF16)
    d["outT"] = nc.dram_tensor("outT", [P, L], F32, kind="ExternalOutput")
    return d


def _body(tc, d):
    nc = tc.nc
    with ExitStack() as ctx:
        const = ctx.enter_context(tc.tile_pool(name="const", bufs=1))

        def cload(name, shape, dt=BF16, src=None):
            t = const.tile(list(shape), dt, tag=name, name=name)
            nc.sync.dma_start(t[:], src if src is not None else d[name][:])
            return t

        w_in = cload("w_in", (P, 4 * P))
        conv_sc = [cload(f"conv_sc{i}", (P, 1), F32, d["conv_sc"][i]) for i in range(DH)]
        conv_bi = [cload(f"conv_bi{i}", (P, 1), F32, d["conv_bi"][i]) for i in range(DH)]
        w_xproj = [[cload(f"w_xproj{k}{i}", (P, 40), BF16, d["w_xproj"][k, i])
                    for i in range(DH)] for k in range(KG)]
        w_dt = [cload(f"w_dt{k}", (DTR, DIN), BF16, d["w_dt"][k]) for k in range(KG)]
        dt_bias = [[cload(f"dt_bias{k}{i}", (P, 1), F32, d["dt_bias"][k, i])
                    for i in range(DH)] for k in range(KG)]
        a_mat = [[cload(f"a_mat{k}{i}", (P, DST), F32, d["a_mat"][k, i])
                  for i in range(DH)] for k in range(KG)]
        diag_ds = [[cload(f"diag_ds{k}{i}", (P, P), BF16, d["diag_ds"][k, i])
                    for i in range(DH)] for k in range(KG)]
        ident = cload("ident", (P, P))
        ones_col = cload("ones_col", (P, 1))
        ones_row = cload("ones_row", (1, P))
        one_f32 = cload("one_f32", (P, 1), F32)
        lnw = [cload(f"lnw{i}", (P, 1), F32, d["lnw"][i]) for i in range(DH)]
        lnb = [cload(f"lnb{i}", (P, 1), F32, d["lnb"][i]) for i in range(DH)]
        w_out = [cload(f"w_out{i}", (P, P), BF16, d["w_out"][i]) for i in range(DH)]
        inv_din = cload("inv_din", (1, 1), F32)
        neg_one = cload("neg_one", (1, 1), F32)
        eps11 = cload("eps11", (1, 1), F32)

        pers = ctx.enter_context(tc.tile_pool(name="pers", bufs=1))
        xh = [pers.tile([P, L], BF16, tag=f"xh{i}", name=f"xh{i}") for i in range(DH)]
        delta = [[pers.tile([P, L], BF16, tag=f"dl{k}{i}", name=f"dl{k}{i}")
                  for i in range(DH)] for k in range(KG)]
        y_sb = [[pers.tile([P, L], BF16, tag=f"y{k}{i}", name=f"y{k}{i}")
                 for i in range(DH)] for k in range(KG)]

        # xs views: k=0 spectral order (hw, t); k=1 reversed natural order.
        def xs_full(i, k):
            if k == 0:
                return xh[i][:].rearrange("p (t hw) -> p hw t", t=T, hw=HWC)
            return xh[i][:, ::-1]

        def xs_chunk(i, k, c):
            if k == 0:
                v = xh[i][:].rearrange("p (t hw) -> p hw t", t=T, hw=HWC)
                nh = MM_F // T  # 32 hw values per 512-col chunk
                return v[:, c * nh:(c + 1) * nh, :]
            hi = L - 1 - c * MM_F
            lo = L - (c + 1) * MM_F - 1
            return xh[i][:, hi:(None if lo < 0 else lo):-1]

        # ================= Phase A: in_proj + depthwise conv + silu ==========
        with tc.tile_pool(name="pA", bufs=1) as pA, \
             tc.tile_pool(name="psA", bufs=4, space=bass.MemorySpace.PSUM) as psA:
            xTb = pA.tile([P, L], BF16, tag="xTb", name="xTb")
            nc.sync.dma_start(xTb[:], d["xTb"][:])
            for po in range(4):
                for c in range(NMM):
                    cs = slice(c * MM_F, (c + 1) * MM_F)
                    ps = psA.tile([P, MM_F], F32, tag="pa", name="pa")
                    nc.tensor.matmul(ps[:], w_in[:, po * P:(po + 1) * P],
                                     xTb[:, cs], start=True, stop=True)
                    if po < DH:
                        nc.scalar.activation(xh[po][:, cs], ps[:], AF.Silu,
                                             bias=conv_bi[po][:], scale=conv_sc[po][:])
                    else:
                        zc = pA.tile([P, MM_F], BF16, tag="zc", name="zc", bufs=3)
                        nc.scalar.activation(zc[:], ps[:], AF.Silu)
                        nc.sync.dma_start(d["z_dram"][po - DH, :, cs], zc[:])

        # ================= Phase B: x_dbl -> B/C spill, delta ================
        with tc.tile_pool(name="pB", bufs=1) as pB, \
             tc.tile_pool(name="psB", bufs=2, space=bass.MemorySpace.PSUM) as psB:
            for k in range(KG):
                xdbl = pB.tile([40, L], BF16, tag="xdbl", name=f"xdbl{k}", bufs=2)
                for c in range(NMM):
                    cs = slice(c * MM_F, (c + 1) * MM_F)
                    ps = psB.tile([40, MM_F], F32, tag="pb", name="pb")
                    nc.tensor.matmul(ps[:], w_xproj[k][0][:], xs_chunk(0, k, c),
                                     start=True, stop=False)
                    nc.tensor.matmul(ps[:], w_xproj[k][1][:], xs_chunk(1, k, c),
                                     start=False, stop=True)
                    nc.scalar.activation(xdbl[:, cs], ps[:], AF.Copy)
                nc.sync.dma_start(
                    d["bc_dram"][k, 0].rearrange("n bc l -> (n bc) l"),
                    xdbl[DTR:40, 0:LC])
                nc.sync.dma_start(
                    d["bc_dram"][k, 1].rearrange("n bc l -> (n bc) l"),
                    xdbl[DTR:40, LC:L])
                for i in range(DH):
                    # softplus(x + dt_b) = ln(1 + exp(x + dt_b)); batch the 8
                    # Exp chunks then one full-width Ln so the Exp/Ln act
                    # tables load once each instead of alternating.
                    ed = pB.tile([P, L], F32, tag="ed", name="ed", bufs=1)
                    for c in range(NMM):
                        cs = slice(c * MM_F, (c + 1) * MM_F)
                        ps2 = psB.tile([P, MM_F], F32, tag="pb2", name="pb2")
                        nc.tensor.matmul(ps2[:], w_dt[k][:, i * P:(i + 1) * P],
                                         xdbl[0:DTR, cs], start=True, stop=True)
                        nc.scalar.activation(ed[:, cs], ps2[:], AF.Exp,
                                             bias=dt_bias[k][i][:])
                    nc.scalar.activation(delta[k][i][:], ed[:], AF.Ln,
                                         bias=one_f32[:])

        # ================= Phase C: selective scan ===========================
        # Half-L slabs: smaller tiles allow deep broadcast prefetch (the
        # full-L version stalled the scan ~10us/iter on brep/crep DMAs), and
        # the two d-halves interleave per n. State chains across halves via
        # the scan's `initial` AP (same cost as a literal initial).
        with tc.tile_pool(name="sc", bufs=2) as sc, \
             tc.tile_pool(name="psC", bufs=1, space=bass.MemorySpace.PSUM) as psC:
            state = sc.tile([P, 2 * DST], F32, tag="state", name="state", bufs=1)
            for k in range(KG):
                dUs = []
                for i in range(DH):
                    dU = sc.tile([P, L], F32 if XIN_ON_POOL else BF16,
                                 tag=f"dU{i}", name=f"dU{i}", bufs=1)
                    if k == 0:
                        nc.vector.tensor_tensor(
                            dU[:].rearrange("p (hw t) -> p hw t", hw=HWC, t=T),
                            delta[k][i][:].rearrange("p (hw t) -> p hw t", hw=HWC, t=T),
                            xs_full(i, k), ALU.mult)
                    else:
                        nc.vector.tensor_tensor(dU[:], delta[k][i][:],
                                                xs_full(i, k), ALU.mult)
                    dUs.append(dU)
                for half in range(2):
                    hs = slice(half * LC, (half + 1) * LC)
                    y_ps = [psC.tile([P, LC], F32, tag=f"yps{i}", name=f"yps{i}")
                            for i in range(DH)]
                    for i in range(DH):
                        for c in range(NMC):
                            nc.tensor.matmul(
                                y_ps[i][:, c * MM_F:(c + 1) * MM_F],
                                diag_ds[k][i][:],
                                xs_chunk(i, k, half * NMC + c),
                                start=True, stop=False)
                    for n in range(DST):
                        bcrep = sc.tile([P, 2 * LC], BF16, tag="bcrep",
                                        name="bcrep", bufs=4)
                        nc.sync.dma_start(
                            bcrep[:],
                            d["bc_dram"][k, half, n:n + 1].partition_broadcast(P))
                        brep = bcrep[:, 0:LC]
                        crep = bcrep[:, LC:2 * LC]
                        for i in range(DH):
                            col = 2 * n + i
                            dA = sc.tile([P, LC], F32, tag="dA", name="dA", bufs=3)
                            nc.scalar.activation(dA[:], delta[k][i][:, hs],
                                                 AF.Exp,
                                                 scale=a_mat[k][i][:, n:n + 1])
                            xin = sc.tile([P, LC], BF16, tag="xin", name="xin")
                            xin_eng = nc.gpsimd if XIN_ON_POOL else nc.vector
                            xin_eng.tensor_tensor(xin[:], dUs[i][:, hs],
                                                  brep, ALU.mult)
                            h = sc.tile([P, LC], BF16, tag="h", name="h")
                            nc.vector.tensor_tensor_scan(
                                h[:], dA[:], xin[:],
                                state[:, col:col + 1] if half == 1 else 0.0,
                                ALU.mult, ALU.add)
                            if half == 0:
                                nc.vector.tensor_copy(state[:, col:col + 1],
                                                      h[:, LC - 1:LC])
                            tmp = sc.tile([P, LC], BF16, tag="tmp", name="tmp", bufs=3)
                            nc.vector.tensor_tensor(tmp[:], crep, h[:],
                                                    ALU.mult)
                            for c in range(NMC):
                                cs = slice(c * MM_F, (c + 1) * MM_F)
                                nc.tensor.matmul(y_ps[i][:, cs], ident[:],
                                                 tmp[:, cs], start=False,
                                                 stop=(n == DST - 1))
                    for i in range(DH):
                        if k == 0:
                            # y_ps cols are (hw, t)-ordered; scatter into the
                            # natural-order y_sb so phase D reads are packed.
                            nhw = LC // T
                            dst = y_sb[k][i][:].rearrange(
                                "p (t hw) -> p hw t", t=T, hw=HWC)[
                                :, half * nhw:(half + 1) * nhw, :]
                            src = y_ps[i][:].rearrange(
                                "p (hw t) -> p hw t", hw=nhw, t=T)
                            nc.scalar.activation(dst, src, AF.Copy)
                        else:
                            nc.scalar.activation(y_sb[k][i][:, hs], y_ps[i][:],
                                                 AF.Copy)

        # ================= Phase D: combine + LN + gate + out_proj ===========
        with tc.tile_pool(name="pD", bufs=1) as pD, \
             tc.tile_pool(name="psD", bufs=2, space=bass.MemorySpace.PSUM) as psD:
            ysum = [pD.tile([P, L], BF16, tag=f"ys{i}", name=f"ys{i}")
                    for i in range(DH)]
            for i in range(DH):
                nc.vector.tensor_tensor(ysum[i][:], y_sb[0][i][:],
                                        y_sb[1][i][:, ::-1], ALU.add)

            for c in range(NMM):
                cs = slice(c * MM_F, (c + 1) * MM_F)
                ps1 = psD.tile([1, MM_F], F32, tag="ps1", name="ps1", bufs=1)
                nc.tensor.matmul(ps1[:], ones_col[:], ysum[0][:, cs],
                                 start=True, stop=False)
                nc.tensor.matmul(ps1[:], ones_col[:], ysum[1][:, cs],
                                 start=False, stop=True)
                ps2 = psD.tile([1, MM_F], F32, tag="ps2", name="ps2", bufs=1)
                for i in range(DH):
                    yq = pD.tile([P, MM_F], BF16, tag="yq", name="yq", bufs=2)
                    nc.scalar.activation(yq[:], ysum[i][:, cs], AF.Square)
                    nc.tensor.matmul(ps2[:], ones_col[:], yq[:],
                                     start=(i == 0), stop=(i == DH - 1))
                mu = pD.tile([1, MM_F], F32, tag="mu", name="mu", bufs=2)
                nc.scalar.activation(mu[:], ps1[:], AF.Identity, scale=inv_din[:])
                e2 = pD.tile([1, MM_F], F32, tag="e2", name="e2", bufs=2)
                nc.scalar.activation(e2[:], ps2[:], AF.Identity, scale=inv_din[:])
                m2 = pD.tile([1, MM_F], F32, tag="m2", name="m2", bufs=2)
                nc.scalar.activation(m2[:], mu[:], AF.Square)
                var = pD.tile([1, MM_F], F32, tag="var", name="var", bufs=2)
                nc.vector.tensor_tensor(var[:], e2[:], m2[:], ALU.subtract)
                sd = pD.tile([1, MM_F], F32, tag="sd", name="sd", bufs=2)
                nc.scalar.activation(sd[:], var[:], AF.Sqrt, bias=eps11[:])
                rr = pD.tile([1, MM_F], F32, tag="rr", name="rr", bufs=2)
                nc.vector.reciprocal_approx_fast(rr[:], sd[:])
                a_row = pD.tile([1, MM_F], BF16, tag="a_row", name="a_row", bufs=2)
                nc.scalar.activation(a_row[:], rr[:], AF.Copy)
                t1 = pD.tile([1, MM_F], F32, tag="t1", name="t1", bufs=2)
                nc.vector.tensor_tensor(t1[:], mu[:], rr[:], ALU.mult)
                b_row = pD.tile([1, MM_F], BF16, tag="b_row", name="b_row", bufs=2)
                nc.scalar.activation(b_row[:], t1[:], AF.Identity, scale=neg_one[:])
                arep = psD.tile([P, MM_F], F32, tag="arep", name="arep")
                nc.tensor.matmul(arep[:], ones_row[:], a_row[:],
                                 start=True, stop=True)
                brep_ln = psD.tile([P, MM_F], F32, tag="brepl", name="brepl")
                nc.tensor.matmul(brep_ln[:], ones_row[:], b_row[:],
                                 start=True, stop=True)
                out_ps = psD.tile([P, MM_F], F32, tag="ops", name="ops")
                for i in range(DH):
                    zc = pD.tile([P, MM_F], BF16, tag="zc2", name="zc2", bufs=3)
                    nc.sync.dma_start(zc[:], d["z_dram"][i, :, cs])
                    yn = pD.tile([P, MM_F], BF16, tag="yn", name="yn", bufs=2)
                    nc.vector.tensor_tensor(yn[:], ysum[i][:, cs], arep[:],
                                            ALU.mult)
                    yn2 = pD.tile([P, MM_F], BF16, tag="yn2", name="yn2", bufs=2)
                    nc.vector.tensor_tensor(yn2[:], yn[:], brep_ln[:], ALU.add)
                    ya = pD.tile([P, MM_F], BF16, tag="ya", name="ya", bufs=2)
                    nc.scalar.activation(ya[:], yn2[:], AF.Identity,
                                         bias=lnb[i][:], scale=lnw[i][:])
                    g = pD.tile([P, MM_F], BF16, tag="g", name="g", bufs=2)
                    nc.vector.tensor_tensor(g[:], ya[:], zc[:], ALU.mult)
                    nc.tensor.matmul(out_ps[:], w_out[i][:], g[:],
                                     start=(i == 0), stop=(i == DH - 1))
                osb = pD.tile([P, MM_F], F32, tag="osb", name="osb", bufs=2)
                nc.scalar.activation(osb[:], out_ps[:], AF.Copy)
                nc.sync.dma_start(d["outT"][:, cs], osb[:])



_CACHE = {}


def _get_program():
    if "nc" not in _CACHE:
        nc = bacc.Bacc("TRN2", target_bir_lowering=False, debug=False,
                       num_devices=NCORES)
        d = _declare_drams(nc)
        with tile.TileContext(nc) as tc:
            _body(tc, d)
        nc.compile()
        _CACHE["nc"] = nc
    return _CACHE["nc"]


# x_proj rows reordered so the B/C rows interleave as (B_n, C_n) pairs, making
# the per-n broadcast source contiguous.
_XPROJ_PERM = list(range(DTR)) + [DTR + 16 * bc + n for n in range(DST) for bc in (0, 1)]


def _host_weights(inputs):
    f32 = lambda a: np.ascontiguousarray(np.asarray(a, np.float32))
    bf = lambda a: np.ascontiguousarray(np.asarray(a, np.float32)).astype(ml_dtypes.bfloat16)
    in_proj_w = f32(inputs["in_proj_w"])            # (512, 128)
    x_proj_w = f32(inputs["x_proj_w"])              # (2, 40, 256)
    dt_w = f32(inputs["dt_w"])                      # (2, 256, 8)
    dt_b = f32(inputs["dt_b"])                      # (2, 256)
    A_logs = f32(inputs["A_logs"])                  # (512, 16)
    Ds = f32(inputs["Ds"])                          # (512,)
    diag_ds = np.zeros((KG, DH, P, P), np.float32)
    for k in range(KG):
        for i in range(DH):
            np.fill_diagonal(diag_ds[k, i], Ds[k * DIN + i * P:k * DIN + (i + 1) * P])
    m = {
        "w_in": bf(in_proj_w.T),
        "conv_sc": f32(inputs["conv_w"]).reshape(DH, P, 1),
        "conv_bi": f32(inputs["conv_b"]).reshape(DH, P, 1),
        "w_xproj": bf(x_proj_w[:, _XPROJ_PERM].transpose(0, 2, 1).reshape(KG, DH, P, 40)),
        "w_dt": bf(dt_w.transpose(0, 2, 1)),
        "dt_bias": f32(dt_b).reshape(KG, DH, P, 1),
        "a_mat": f32(-np.exp(A_logs)).reshape(KG, DH, P, DST),
        "diag_ds": diag_ds.astype(ml_dtypes.bfloat16),
        "ident": np.eye(P, dtype=np.float32).astype(ml_dtypes.bfloat16),
        "ones_col": np.ones((P, 1), np.float32).astype(ml_dtypes.bfloat16),
        "ones_row": np.ones((1, P), np.float32).astype(ml_dtypes.bfloat16),
        "one_f32": np.ones((P, 1), np.float32),
        "lnw": f32(inputs["ln_w"]).reshape(DH, P, 1),
        "lnb": f32(inputs["ln_b"]).reshape(DH, P, 1),
        "w_out": bf(f32(inputs["out_proj_w"]).T.reshape(DH, P, P)),
        "inv_din": np.full((1, 1), 1.0 / DIN, np.float32),
        "neg_one": np.full((1, 1), -1.0, np.float32),
        "eps11": np.full((1, 1), 1e-5, np.float32),
    }
    return m


def kernel(**inputs):
    x = np.ascontiguousarray(np.asarray(inputs["x"], np.float32))   # (8,16,16,16,128)
    shared = _host_weights(inputs)
    nc = _get_program()
    in_maps = []
    for b in range(NCORES):
        m = dict(shared)
        m["xTb"] = np.ascontiguousarray(
            x[b].reshape(L, DIM).T).astype(ml_dtypes.bfloat16)
        in_maps.append(m)
    trace = bool(int(os.environ.get("BASS_PROFILE", "0")))
    res = run_bass_kernel_spmd(nc, in_maps, list(range(NCORES)), trace=trace)
    _CACHE["last_result"] = res
    outs = [np.asarray(r["outT"], np.float32) for r in res.results]
    out = np.stack([o.T.reshape(T, H, W, DIM) for o in outs]).astype(np.float32)
    return out


# revision 20
# speedup vs baseline: 1.0219x; 1.0017x over previous
"""Trainium2 Bass kernel for nn_Block_26628797235524 (Mamba-style cross-scan SSM block).

Sharding: batch B=8 -> one batch element per NeuronCore (SPMD, no collectives).

v2 design (vs v1 baseline at ~2.2ms):
  - bf16 dataflow everywhere precision allows (matmuls, elementwise, scan
    inputs/outputs); the scan's dA decay factor stays fp32 (tensor_tensor_scan
    keeps fp32 internal state, and exp(A*delta)~1 would be wrecked by bf16).
  - Silu activation function on the Scalar engine (1 op instead of
    sigmoid+2 vector multiplies).
  - Full-L (4096) scans: no slab chaining, no state copies.
  - Skip connection Ds*xs injected as a diag(Ds) matmul directly into the
    PSUM y accumulator (start=True), freeing the vector engine.
  - xs orderings are APs (strided / reversed views of xh), never materialized.
  - z and B/C rows spilled to DRAM in bf16; B/C broadcast via DMA
    partition_broadcast per state-dim n.
  - All elementwise scan work on DVE (Pool's bf16 reads are slow and it
    shares SBUF ports with DVE); Scalar does all exp/silu/softplus/copies.
"""

import os
import sys
from contextlib import ExitStack

import numpy as np
import ml_dtypes

_RL = "/opt/trn_rl_repo"
if os.path.isdir(_RL) and _RL not in sys.path:
    sys.path.insert(0, _RL)

import concourse.bass as bass
import concourse.bacc as bacc
import concourse.tile as tile
from concourse import mybir
from concourse.bass_utils import run_bass_kernel_spmd

B, T, H, W, DIM = 8, 16, 16, 16, 128
DIN, DST, DTR, KG = 256, 16, 8, 2
L = T * H * W          # 4096
P = 128
HWC = H * W            # 256
DH = DIN // P          # 2
NCORES = 8
MM_F = 512             # PSUM bank: max matmul free dim (fp32 out)
NMM = L // MM_F        # 8
LC = L // 2            # scan slab length
NMC = LC // MM_F       # 4

F32 = mybir.dt.float32
BF16 = mybir.dt.bfloat16
XIN_ON_POOL = bool(int(os.environ.get("XIN_ON_POOL", "0")))
AF = mybir.ActivationFunctionType
ALU = mybir.AluOpType


def _declare_drams(nc):
    d = {}

    def inp(name, shape, dt=BF16):
        d[name] = nc.dram_tensor(name, list(shape), dt, kind="ExternalInput")

    inp("xTb", (P, L))                      # per-core x, channel-major, bf16
    inp("w_in", (P, 4 * P))                 # in_proj_w.T
    inp("conv_sc", (DH, P, 1), F32)
    inp("conv_bi", (DH, P, 1), F32)
    inp("w_xproj", (KG, DH, P, 40))         # x_proj_w[k].T per d-half
    inp("w_dt", (KG, DTR, DIN))             # dt_w[k].T
    inp("dt_bias", (KG, DH, P, 1), F32)
    inp("a_mat", (KG, DH, P, DST), F32)     # A = -exp(A_logs)
    inp("diag_ds", (KG, DH, P, P))          # diag(Ds) per (k, d-half)
    inp("ident", (P, P))
    inp("ones_col", (P, 1))
    inp("ones_row", (1, P))
    inp("one_f32", (P, 1), F32)
    inp("lnw", (DH, P, 1), F32)
    inp("lnb", (DH, P, 1), F32)
    inp("w_out", (DH, P, P))                # out_proj_w.T per d-half
    inp("inv_din", (1, 1), F32)             # 1/256
    inp("neg_one", (1, 1), F32)
    inp("eps11", (1, 1), F32)
    d["z_dram"] = nc.dram_tensor("z_dram", [DH, P, L], BF16)
    d["bc_dram"] = nc.dram_tensor("bc_dram", [KG, 2, DST, 2, LC], BF16)
    d["outT"] = nc.dram_tensor("outT", [P, L], F32, kind="ExternalOutput")
    return d


def _body(tc, d):
    nc = tc.nc
    with ExitStack() as ctx:
        const = ctx.enter_context(tc.tile_pool(name="const", bufs=1))

        def cload(name, shape, dt=BF16, src=None):
            t = const.tile(list(shape), dt, tag=name, name=name)
            nc.sync.dma_start(t[:], src if src is not None else d[name][:])
            return t

        w_in = cload("w_in", (P, 4 * P))
        conv_sc = [cload(f"conv_sc{i}", (P, 1), F32, d["conv_sc"][i]) for i in range(DH)]
        conv_bi = [cload(f"conv_bi{i}", (P, 1), F32, d["conv_bi"][i]) for i in range(DH)]
        w_xproj = [[cload(f"w_xproj{k}{i}", (P, 40), BF16, d["w_xproj"][k, i])
                    for i in range(DH)] for k in range(KG)]
        w_dt = [cload(f"w_dt{k}", (DTR, DIN), BF16, d["w_dt"][k]) for k in range(KG)]
        dt_bias = [[cload(f"dt_bias{k}{i}", (P, 1), F32, d["dt_bias"][k, i])
                    for i in range(DH)] for k in range(KG)]
        a_mat = [[cload(f"a_mat{k}{i}", (P, DST), F32, d["a_mat"][k, i])
                  for i in range(DH)] for k in range(KG)]
        diag_ds = [[cload(f"diag_ds{k}{i}", (P, P), BF16, d["diag_ds"][k, i])
                    for i in range(DH)] for k in range(KG)]
        ident = cload("ident", (P, P))
        ones_col = cload("ones_col", (P, 1))
        ones_row = cload("ones_row", (1, P))
        one_f32 = cload("one_f32", (P, 1), F32)
        lnw = [cload(f"lnw{i}", (P, 1), F32, d["lnw"][i]) for i in range(DH)]
        lnb = [cload(f"lnb{i}", (P, 1), F32, d["lnb"][i]) for i in range(DH)]
        w_out = [cload(f"w_out{i}", (P, P), BF16, d["w_out"][i]) for i in range(DH)]
        inv_din = cload("inv_din", (1, 1), F32)
        neg_one = cload("neg_one", (1, 1), F32)
        eps11 = cload("eps11", (1, 1), F32)

        pers = ctx.enter_context(tc.tile_pool(name="pers", bufs=1))
        xh = [pers.tile([P, L], BF16, tag=f"xh{i}", name=f"xh{i}") for i in range(DH)]
        delta = [[pers.tile([P, L], BF16, tag=f"dl{k}{i}", name=f"dl{k}{i}")
                  for i in range(DH)] for k in range(KG)]
        y_sb = [[pers.tile([P, L], BF16, tag=f"y{k}{i}", name=f"y{k}{i}")
                 for i in range(DH)] for k in range(KG)]

        # xs views: k=0 spectral order (hw, t); k=1 reversed natural order.
        def xs_full(i, k):
            if k == 0:
                return xh[i][:].rearrange("p (t hw) -> p hw t", t=T, hw=HWC)
            return xh[i][:, ::-1]

        def xs_chunk(i, k, c):
            if k == 0:
                v = xh[i][:].rearrange("p (t hw) -> p hw t", t=T, hw=HWC)
                nh = MM_F // T  # 32 hw values per 512-col chunk
                return v[:, c * nh:(c + 1) * nh, :]
            hi = L - 1 - c * MM_F
            lo = L - (c + 1) * MM_F - 1
            return xh[i][:, hi:(None if lo < 0 else lo):-1]

        # ================= Phase A: in_proj + depthwise conv + silu ==========
        with tc.tile_pool(name="pA", bufs=1) as pA, \
             tc.tile_pool(name="psA", bufs=4, space=bass.MemorySpace.PSUM) as psA:
            xTb = pA.tile([P, L], BF16, tag="xTb", name="xTb")
            nc.sync.dma_start(xTb[:], d["xTb"][:])
            for po in range(4):
                for c in range(NMM):
                    cs = slice(c * MM_F, (c + 1) * MM_F)
                    ps = psA.tile([P, MM_F], F32, tag="pa", name="pa")
                    nc.tensor.matmul(ps[:], w_in[:, po * P:(po + 1) * P],
                                     xTb[:, cs], start=True, stop=True)
                    if po < DH:
                        nc.scalar.activation(xh[po][:, cs], ps[:], AF.Silu,
                                             bias=conv_bi[po][:], scale=conv_sc[po][:])
                    else:
                        zc = pA.tile([P, MM_F], BF16, tag="zc", name="zc", bufs=3)
                        nc.scalar.activation(zc[:], ps[:], AF.Silu)
                        nc.sync.dma_start(d["z_dram"][po - DH, :, cs], zc[:])

        # ================= Phase B: x_dbl -> B/C spill, delta ================
        with tc.tile_pool(name="pB", bufs=1) as pB, \
             tc.tile_pool(name="psB", bufs=2, space=bass.MemorySpace.PSUM) as psB:
            for k in range(KG):
                xdbl = pB.tile([40, L], BF16, tag="xdbl", name=f"xdbl{k}", bufs=2)
                for c in range(NMM):
                    cs = slice(c * MM_F, (c + 1) * MM_F)
                    ps = psB.tile([40, MM_F], F32, tag="pb", name="pb")
                    nc.tensor.matmul(ps[:], w_xproj[k][0][:], xs_chunk(0, k, c),
                                     start=True, stop=False)
                    nc.tensor.matmul(ps[:], w_xproj[k][1][:], xs_chunk(1, k, c),
                                     start=False, stop=True)
                    nc.scalar.activation(xdbl[:, cs], ps[:], AF.Copy)
                nc.sync.dma_start(
                    d["bc_dram"][k, 0].rearrange("n bc l -> (n bc) l"),
                    xdbl[DTR:40, 0:LC])
                nc.sync.dma_start(
                    d["bc_dram"][k, 1].rearrange("n bc l -> (n bc) l"),
                    xdbl[DTR:40, LC:L])
                for i in range(DH):
                    # softplus(x + dt_b) = ln(1 + exp(x + dt_b)); batch the 8
                    # Exp chunks then one full-width Ln so the Exp/Ln act
                    # tables load once each instead of alternating.
                    ed = pB.tile([P, L], F32, tag="ed", name="ed", bufs=1)
                    for c in range(NMM):
                        cs = slice(c * MM_F, (c + 1) * MM_F)
                        ps2 = psB.tile([P, MM_F], F32, tag="pb2", name="pb2")
                        nc.tensor.matmul(ps2[:], w_dt[k][:, i * P:(i + 1) * P],
                                         xdbl[0:DTR, cs], start=True, stop=True)
                        nc.scalar.activation(ed[:, cs], ps2[:], AF.Exp,
                                             bias=dt_bias[k][i][:])
                    nc.scalar.activation(delta[k][i][:], ed[:], AF.Ln,
                                         bias=one_f32[:])

        # ================= Phase C: selective scan ===========================
        # Half-L slabs: smaller tiles allow deep broadcast prefetch (the
        # full-L version stalled the scan ~10us/iter on brep/crep DMAs), and
        # the two d-halves interleave per n. State chains across halves via
        # the scan's `initial` AP (same cost as a literal initial).
        with tc.tile_pool(name="sc", bufs=2) as sc, \
             tc.tile_pool(name="psC", bufs=1, space=bass.MemorySpace.PSUM) as psC:
            state = sc.tile([P, 2 * DST], F32, tag="state", name="state", bufs=1)
            for k in range(KG):
                dUs = []
                for i in range(DH):
                    dU = sc.tile([P, L], F32 if XIN_ON_POOL else BF16,
                                 tag=f"dU{i}", name=f"dU{i}", bufs=1)
                    if k == 0:
                        nc.vector.tensor_tensor(
                            dU[:].rearrange("p (hw t) -> p hw t", hw=HWC, t=T),
                            delta[k][i][:].rearrange("p (hw t) -> p hw t", hw=HWC, t=T),
                            xs_full(i, k), ALU.mult)
                    else:
                        nc.vector.tensor_tensor(dU[:], delta[k][i][:],
                                                xs_full(i, k), ALU.mult)
                    dUs.append(dU)
                for half in range(2):
                    hs = slice(half * LC, (half + 1) * LC)
                    y_ps = [psC.tile([P, LC], F32, tag=f"yps{i}", name=f"yps{i}")
                            for i in range(DH)]
                    for i in range(DH):
                        for c in range(NMC):
                            nc.tensor.matmul(
                                y_ps[i][:, c * MM_F:(c + 1) * MM_F],
                                diag_ds[k][i][:],
                                xs_chunk(i, k, half * NMC + c),
                                start=True, stop=False)
                    for n in range(DST):
                        bcrep = sc.tile([P, 2 * LC], BF16, tag="bcrep",
                                        name="bcrep", bufs=4)
                        nc.sync.dma_start(
                            bcrep[:],
                            d["bc_dram"][k, half, n:n + 1].partition_broadcast(P))
                        brep = bcrep[:, 0:LC]
                        crep = bcrep[:, LC:2 * LC]
                        for i in range(DH):
                            col = 2 * n + i
                            dA = sc.tile([P, LC], F32, tag="dA", name="dA", bufs=3)
                            nc.scalar.activation(dA[:], delta[k][i][:, hs],
                                                 AF.Exp,
                                                 scale=a_mat[k][i][:, n:n + 1])
                            xin = sc.tile([P, LC], BF16, tag="xin", name="xin")
                            xin_eng = nc.gpsimd if XIN_ON_POOL else nc.vector
                            xin_eng.tensor_tensor(xin[:], dUs[i][:, hs],
                                                  brep, ALU.mult)
                            h = sc.tile([P, LC], BF16, tag="h", name="h")
                            nc.vector.tensor_tensor_scan(
                                h[:], dA[:], xin[:],
                                state[:, col:col + 1] if half == 1 else 0.0,
                                ALU.mult, ALU.add)
                            if half == 0:
                                nc.vector.tensor_copy(state[:, col:col + 1],
                                                      h[:, LC - 1:LC])
                            tmp = sc.tile([P, LC], BF16, tag="tmp", name="tmp", bufs=3)
                            nc.vector.tensor_tensor(tmp[:], crep, h[:],
                                                    ALU.mult)
                            for c in range(NMC):
                                cs = slice(c * MM_F, (c + 1) * MM_F)
                                nc.tensor.matmul(y_ps[i][:, cs], ident[:],
                                                 tmp[:, cs], start=False,
                                                 stop=(n == DST - 1))
                    for i in range(DH):
                        if k == 0:
                            # y_ps cols are (hw, t)-ordered; scatter into the
                            # natural-order y_sb so phase D reads are packed.
                            nhw = LC // T
                            dst = y_sb[k][i][:].rearrange(
                                "p (t hw) -> p hw t", t=T, hw=HWC)[
                                :, half * nhw:(half + 1) * nhw, :]
                            src = y_ps[i][:].rearrange(
                                "p (hw t) -> p hw t", hw=nhw, t=T)
                            nc.scalar.activation(dst, src, AF.Copy)
                        else:
                            nc.scalar.activation(y_sb[k][i][:, hs], y_ps[i][:],
                                                 AF.Copy)

        # ================= Phase D: combine + LN + gate + out_proj ===========
        with tc.tile_pool(name="pD", bufs=1) as pD, \
             tc.tile_pool(name="psD", bufs=2, space=bass.MemorySpace.PSUM) as psD:
            ysum = [pD.tile([P, L], BF16, tag=f"ys{i}", name=f"ys{i}")
                    for i in range(DH)]
            for c in range(NMM):
                cs = slice(c * MM_F, (c + 1) * MM_F)
                rcs = slice(L - (c + 1) * MM_F, L - c * MM_F)
                for i in range(DH):
                    nc.vector.tensor_tensor(
                        ysum[i][:, cs], y_sb[0][i][:, cs],
                        y_sb[1][i][:, rcs][:, ::-1], ALU.add)
                ps1 = psD.tile([1, MM_F], F32, tag="ps1", name="ps1", bufs=1)
                nc.tensor.matmul(ps1[:], ones_col[:], ysum[0][:, cs],
                                 start=True, stop=False)
                nc.tensor.matmul(ps1[:], ones_col[:], ysum[1][:, cs],
                                 start=False, stop=True)
                ps2 = psD.tile([1, MM_F], F32, tag="ps2", name="ps2", bufs=1)
                for i in range(DH):
                    yq = pD.tile([P, MM_F], BF16, tag="yq", name="yq", bufs=2)
                    nc.scalar.activation(yq[:], ysum[i][:, cs], AF.Square)
                    nc.tensor.matmul(ps2[:], ones_col[:], yq[:],
                                     start=(i == 0), stop=(i == DH - 1))
                mu = pD.tile([1, MM_F], F32, tag="mu", name="mu", bufs=2)
                nc.scalar.activation(mu[:], ps1[:], AF.Identity, scale=inv_din[:])
                e2 = pD.tile([1, MM_F], F32, tag="e2", name="e2", bufs=2)
                nc.scalar.activation(e2[:], ps2[:], AF.Identity, scale=inv_din[:])
                m2 = pD.tile([1, MM_F], F32, tag="m2", name="m2", bufs=2)
                nc.scalar.activation(m2[:], mu[:], AF.Square)
                var = pD.tile([1, MM_F], F32, tag="var", name="var", bufs=2)
                nc.vector.tensor_tensor(var[:], e2[:], m2[:], ALU.subtract)
                sd = pD.tile([1, MM_F], F32, tag="sd", name="sd", bufs=2)
                nc.scalar.activation(sd[:], var[:], AF.Sqrt, bias=eps11[:])
                rr = pD.tile([1, MM_F], F32, tag="rr", name="rr", bufs=2)
                nc.vector.reciprocal_approx_fast(rr[:], sd[:])
                a_row = pD.tile([1, MM_F], BF16, tag="a_row", name="a_row", bufs=2)
                nc.scalar.activation(a_row[:], rr[:], AF.Copy)
                t1 = pD.tile([1, MM_F], F32, tag="t1", name="t1", bufs=2)
                nc.vector.tensor_tensor(t1[:], mu[:], rr[:], ALU.mult)
                b_row = pD.tile([1, MM_F], BF16, tag="b_row", name="b_row", bufs=2)
                nc.scalar.activation(b_row[:], t1[:], AF.Identity, scale=neg_one[:])
                arep = psD.tile([P, MM_F], F32, tag="arep", name="arep")
                nc.tensor.matmul(arep[:], ones_row[:], a_row[:],
                                 start=True, stop=True)
                brep_ln = psD.tile([P, MM_F], F32, tag="brepl", name="brepl")
                nc.tensor.matmul(brep_ln[:], ones_row[:], b_row[:],
                                 start=True, stop=True)
                out_ps = psD.tile([P, MM_F], F32, tag="ops", name="ops")
                for i in range(DH):
                    zc = pD.tile([P, MM_F], BF16, tag="zc2", name="zc2", bufs=3)
                    nc.sync.dma_start(zc[:], d["z_dram"][i, :, cs])
                    yn = pD.tile([P, MM_F], BF16, tag="yn", name="yn", bufs=2)
                    nc.vector.tensor_tensor(yn[:], ysum[i][:, cs], arep[:],
                                            ALU.mult)
                    yn2 = pD.tile([P, MM_F], BF16, tag="yn2", name="yn2", bufs=2)
                    nc.vector.tensor_tensor(yn2[:], yn[:], brep_ln[:], ALU.add)
                    ya = pD.tile([P, MM_F], BF16, tag="ya", name="ya", bufs=2)
                    nc.scalar.activation(ya[:], yn2[:], AF.Identity,
                                         bias=lnb[i][:], scale=lnw[i][:])
                    g = pD.tile([P, MM_F], BF16, tag="g", name="g", bufs=2)
                    nc.vector.tensor_tensor(g[:], ya[:], zc[:], ALU.mult)
                    nc.tensor.matmul(out_ps[:], w_out[i][:], g[:],
                                     start=(i == 0), stop=(i == DH - 1))
                osb = pD.tile([P, MM_F], F32, tag="osb", name="osb", bufs=2)
                nc.scalar.activation(osb[:], out_ps[:], AF.Copy)
                nc.sync.dma_start(d["outT"][:, cs], osb[:])



_CACHE = {}


def _get_program():
    if "nc" not in _CACHE:
        nc = bacc.Bacc("TRN2", target_bir_lowering=False, debug=False,
                       num_devices=NCORES)
        d = _declare_drams(nc)
        with tile.TileContext(nc) as tc:
            _body(tc, d)
        nc.compile()
        _CACHE["nc"] = nc
    return _CACHE["nc"]


# x_proj rows reordered so the B/C rows interleave as (B_n, C_n) pairs, making
# the per-n broadcast source contiguous.
_XPROJ_PERM = list(range(DTR)) + [DTR + 16 * bc + n for n in range(DST) for bc in (0, 1)]


def _host_weights(inputs):
    f32 = lambda a: np.ascontiguousarray(np.asarray(a, np.float32))
    bf = lambda a: np.ascontiguousarray(np.asarray(a, np.float32)).astype(ml_dtypes.bfloat16)
    in_proj_w = f32(inputs["in_proj_w"])            # (512, 128)
    x_proj_w = f32(inputs["x_proj_w"])              # (2, 40, 256)
    dt_w = f32(inputs["dt_w"])                      # (2, 256, 8)
    dt_b = f32(inputs["dt_b"])                      # (2, 256)
    A_logs = f32(inputs["A_logs"])                  # (512, 16)
    Ds = f32(inputs["Ds"])                          # (512,)
    diag_ds = np.zeros((KG, DH, P, P), np.float32)
    for k in range(KG):
        for i in range(DH):
            np.fill_diagonal(diag_ds[k, i], Ds[k * DIN + i * P:k * DIN + (i + 1) * P])
    m = {
        "w_in": bf(in_proj_w.T),
        "conv_sc": f32(inputs["conv_w"]).reshape(DH, P, 1),
        "conv_bi": f32(inputs["conv_b"]).reshape(DH, P, 1),
        "w_xproj": bf(x_proj_w[:, _XPROJ_PERM].transpose(0, 2, 1).reshape(KG, DH, P, 40)),
        "w_dt": bf(dt_w.transpose(0, 2, 1)),
        "dt_bias": f32(dt_b).reshape(KG, DH, P, 1),
        "a_mat": f32(-np.exp(A_logs)).reshape(KG, DH, P, DST),
        "diag_ds": diag_ds.astype(ml_dtypes.bfloat16),
        "ident": np.eye(P, dtype=np.float32).astype(ml_dtypes.bfloat16),
        "ones_col": np.ones((P, 1), np.float32).astype(ml_dtypes.bfloat16),
        "ones_row": np.ones((1, P), np.float32).astype(ml_dtypes.bfloat16),
        "one_f32": np.ones((P, 1), np.float32),
        "lnw": f32(inputs["ln_w"]).reshape(DH, P, 1),
        "lnb": f32(inputs["ln_b"]).reshape(DH, P, 1),
        "w_out": bf(f32(inputs["out_proj_w"]).T.reshape(DH, P, P)),
        "inv_din": np.full((1, 1), 1.0 / DIN, np.float32),
        "neg_one": np.full((1, 1), -1.0, np.float32),
        "eps11": np.full((1, 1), 1e-5, np.float32),
    }
    return m


def kernel(**inputs):
    x = np.ascontiguousarray(np.asarray(inputs["x"], np.float32))   # (8,16,16,16,128)
    shared = _host_weights(inputs)
    nc = _get_program()
    in_maps = []
    for b in range(NCORES):
        m = dict(shared)
        m["xTb"] = np.ascontiguousarray(
            x[b].reshape(L, DIM).T).astype(ml_dtypes.bfloat16)
        in_maps.append(m)
    trace = bool(int(os.environ.get("BASS_PROFILE", "0")))
    res = run_bass_kernel_spmd(nc, in_maps, list(range(NCORES)), trace=trace)
    _CACHE["last_result"] = res
    outs = [np.asarray(r["outT"], np.float32) for r in res.results]
    out = np.stack([o.T.reshape(T, H, W, DIM) for o in outs]).astype(np.float32)
    return out
